# revision 1
# baseline (speedup 1.0000x reference)
"""Trainium2 kernel for nn_DifferentiableModalPlate.

displacement[n] = sum_m P_m * exp(-sigma_m*(n-1)*K) * sin(n*omega_m*K) / (sin(omega_m*K)+1e-8)

Each mode is a damped sinusoid Im(A_m * z_m^n) with z_m = r_m*e^{i w_m}.
Splitting n = t*B + j turns the [modes, N] synthesis + mode-reduction into
a single matmul  Y[T, B] = Ut[K, T].T @ W[K, B]  with K = 2*modes rows
(sin/cos pairs):
    Y[t, j] = sum_m  u_m(t)*S_m(j) + v_m(t)*C_m(j)
    u_m(t) = A_m r^(tB) cos(w tB)   S_m(j) = r^j sin(w j)
    v_m(t) = A_m r^(tB) sin(w tB)   C_m(j) = r^j cos(w j)

Mode axis is sharded across 8 NeuronCores; each core runs K/128
PSUM-accumulated matmuls; partial [T, B] outputs are summed on host and
normalized by the peak.
"""

import math

import numpy as np

import concourse.tile as tile
from concourse import bacc, mybir
from concourse.bass_utils import run_bass_kernel_spmd

N_CORES = 8
SAMPLE_RATE = 44100
K_DT = 1.0 / SAMPLE_RATE
MAX_OM = 10000.0 * 2.0 * np.pi
MIN_OM = 20.0 * 2.0 * np.pi
LX = 0.5
TAU0, TAU1 = 6.0, 1.0
_OM2 = 2.0 * np.pi * 500.0
_DOMSQ = _OM2 ** 2
ALPHA = float(np.float32(3.0 * np.log(10.0) / _DOMSQ * (_OM2 ** 2 / TAU0)))
BETA = float(np.float32(3.0 * np.log(10.0) / _DOMSQ * (1.0 / TAU1 - 1.0 / TAU0)))
M_MAX = N_MAX = 80
_gm, _gn = np.meshgrid(np.arange(1, M_MAX + 1), np.arange(1, N_MAX + 1), indexing="ij")
M_VEC = _gm.reshape(-1).astype(np.float64)
N_VEC = _gn.reshape(-1).astype(np.float64)

# Exposed for test harness introspection (exec_time_ns etc.)
LAST_RESULTS = None


def _softplus(x):
    return np.logaddexp(x, 0.0)


def _mode_params(mu_raw, D_over_mu_raw, T0_over_mu_raw, Ly_raw, xo_raw, yo_raw):
    """Per-mode amplitude A, decay rate r = exp(-sigma*K), phase step w = omega*K (f64)."""
    mu = _softplus(mu_raw) + 1e-4
    D_over_mu = _softplus(D_over_mu_raw) + 1e-4
    T0_over_mu = _softplus(T0_over_mu_raw) + 1e-4
    Ly = 1.1 + (4.0 - 1.1) * ((np.tanh(Ly_raw) + 1.0) / 2.0)
    xo = 0.49 * LX + (1.0 - 0.49) * LX * ((np.tanh(xo_raw) + 1.0) / 2.0)
    yo = 0.51 * Ly + (1.0 - 0.51) * Ly * ((np.tanh(yo_raw) + 1.0) / 2.0)
    xi = 0.1 * LX
    yi = 0.1 * Ly

    pi = np.pi
    g1 = (M_VEC * pi / LX) ** 2 + (N_VEC * pi / Ly) ** 2
    omega = np.sqrt(np.maximum(T0_over_mu * g1 + D_over_mu * g1 * g1, 0.0))
    valid = (omega <= MAX_OM) & (omega >= MIN_OM)

    in_w = np.cos(xi * pi * M_VEC / LX) * np.cos(yi * pi * N_VEC / Ly)
    out_w = np.cos(xo * pi * M_VEC / LX) * np.cos(yo * pi * N_VEC / Ly)
    sigma = ALPHA + BETA * omega ** 2
    ms = 0.25 * mu * LX * Ly
    P = out_w * in_w * (K_DT ** 2) * np.exp(-sigma * K_DT) / ms * valid

    keep = P != 0.0
    P, omega, sigma = P[keep], omega[keep], sigma[keep]
    A = P * np.exp(sigma * K_DT) / (np.sin(omega * K_DT) + 1e-8)
    w = omega * K_DT
    neg_sk = -sigma * K_DT  # log(r)

    # Drop negligible modes: the smallest-|A| set whose summed amplitude is
    # below 1e-5 * max|A| can shift the output by at most ~1e-5 of the peak
    # (peak >= O(max|A|)), far under the fp16 noise floor already accepted.
    if A.shape[0]:
        order = np.argsort(np.abs(A))
        cum = np.cumsum(np.abs(A)[order])
        ndrop = int(np.searchsorted(cum, 1e-5 * np.abs(A).max()))
        if ndrop:
            kept = np.sort(order[ndrop:])
            A, neg_sk, w = A[kept], neg_sk[kept], w[kept]
    return A, neg_sk, w


_PROGRAM_CACHE = {}


def _build_program(kc, t_dim, b_dim):
    """Bass program: Y[t_dim, b_dim] = sum_ki UW[:, ki, :t].T @ UW[:, ki, t:].

    States (first t_dim cols) and tables (last b_dim cols) are packed into one
    fp16 chunk-major tensor [128, n_chunks, t_dim+b_dim] so each load covers
    long per-partition-contiguous runs (full HBM burst efficiency).
    """
    nc = bacc.Bacc(
        "TRN2",
        target_bir_lowering=False,
        debug=False,
        enable_asserts=False,
        enable_partition_id=False,
        num_devices=N_CORES,
    )
    f32 = mybir.dt.float32
    f16 = mybir.dt.float16
    n_chunks = kc // 128
    uw_d = nc.dram_tensor(
        "uw", [128, n_chunks, t_dim + b_dim], f16, kind="ExternalInput"
    )
    y_d = nc.dram_tensor("y", [t_dim, b_dim], f32, kind="ExternalOutput")

    # At most one chunk group per DMA-capable engine, streamed on independent
    # DGE queues so the loads run in parallel and matmuls start as soon as
    # their group has landed.
    dma_engines = [nc.sync, nc.scalar, nc.gpsimd]
    per = max(2, -(-n_chunks // len(dma_engines)))
    groups = []
    pos = 0
    while pos < n_chunks:
        groups.append((pos, min(pos + per, n_chunks)))
        pos += per

    with tile.TileContext(nc) as tc:
        with (
            tc.tile_pool(name="pin", bufs=1) as pin,
            tc.tile_pool(name="pps", bufs=1, space="PSUM") as pps,
        ):
            acc = pps.tile([t_dim, b_dim], f32)
            tiles = []
            for gi, (a, b) in enumerate(groups):
                gt = pin.tile([128, b - a, t_dim + b_dim], f16, tag=f"g{gi}")
                dma_engines[gi % len(dma_engines)].dma_start(
                    out=gt[:], in_=uw_d[:, a:b, :]
                )
                tiles.append(gt)
            for gi, (a, b) in enumerate(groups):
                for ki in range(b - a):
                    nc.tensor.matmul(
                        acc[:],
                        tiles[gi][:, ki, 0:t_dim],
                        tiles[gi][:, ki, t_dim:t_dim + b_dim],
                        start=(gi == 0 and ki == 0),
                        stop=(gi == len(groups) - 1 and ki == b - a - 1),
                    )
            y_t = pin.tile([t_dim, b_dim], f32, tag="yout")
            nc.vector.tensor_copy(y_t[:], acc[:])
            nc.sync.dma_start(out=y_d[:], in_=y_t[:])
    nc.compile()
    return nc


def kernel(mu_raw, D_over_mu_raw, T0_over_mu_raw, Ly_raw, xo_raw, yo_raw, num_samples):
    global LAST_RESULTS
    n = int(num_samples)
    A, neg_sk, w = _mode_params(
        float(mu_raw), float(D_over_mu_raw), float(T0_over_mu_raw),
        float(Ly_raw), float(xo_raw), float(yo_raw),
    )
    nv = A.shape[0]
    if nv == 0 or n == 0:
        return np.zeros(n, dtype=np.float32)

    # Block decomposition: n = t*B + j, T <= 128 (PSUM partitions), B <= 512 (bank).
    b_dim = max(1, math.ceil(n / 128))
    t_dim = math.ceil(n / b_dim)
    assert b_dim <= 512 and t_dim <= 128, (t_dim, b_dim)

    mc = math.ceil(nv / N_CORES)          # modes per core
    kc = ((2 * mc + 127) // 128) * 128    # K rows per core, padded

    # f64 tables/states for all valid modes at once.
    jj = np.arange(b_dim, dtype=np.float64)
    tt = np.arange(t_dim, dtype=np.float64) * b_dim
    decay_j = np.exp(np.outer(neg_sk, jj))        # [nv, B]
    phase_j = np.outer(w, jj)
    S = (decay_j * np.sin(phase_j)).astype(np.float32)
    C = (decay_j * np.cos(phase_j)).astype(np.float32)
    decay_t = A[:, None] * np.exp(np.outer(neg_sk, tt))  # [nv, T]
    phase_t = np.outer(w, tt)
    U = (decay_t * np.cos(phase_t)).astype(np.float32)
    V = (decay_t * np.sin(phase_t)).astype(np.float32)

    # Global power-of-2 scale so fp16 states stay normal (range ~2e-5 raw).
    # The scale divides out of the partial sums before normalization.
    m_abs = max(np.abs(U).max(), np.abs(V).max(), 1e-300)
    scale = 2.0 ** np.floor(np.log2(16384.0 / m_abs))
    U16 = (U * scale).astype(np.float16)
    V16 = (V * scale).astype(np.float16)
    S16 = S.astype(np.float16)
    C16 = C.astype(np.float16)

    n_chunks = kc // 128
    in_maps = []
    for c in range(N_CORES):
        lo, hi = c * mc, min((c + 1) * mc, nv)
        m = hi - lo
        ut = np.zeros((kc, t_dim), dtype=np.float16)
        wt = np.zeros((kc, b_dim), dtype=np.float16)
        if m > 0:
            ut[:m] = U16[lo:hi]
            ut[mc:mc + m] = V16[lo:hi]
            wt[:m] = S16[lo:hi]
            wt[mc:mc + m] = C16[lo:hi]
        # chunk-major pack: [128, n_chunks, t_dim+b_dim], row k=ki*128+p -> [p, ki, :]
        uw = np.concatenate(
            [ut.reshape(n_chunks, 128, t_dim), wt.reshape(n_chunks, 128, b_dim)],
            axis=2,
        ).transpose(1, 0, 2)
        in_maps.append({"uw": np.ascontiguousarray(uw)})

    key = (kc, t_dim, b_dim)
    if key not in _PROGRAM_CACHE:
        _PROGRAM_CACHE[key] = _build_program(*key)
    nc = _PROGRAM_CACHE[key]

    res = run_bass_kernel_spmd(nc, in_maps, core_ids=list(range(N_CORES)))
    LAST_RESULTS = res

    total = np.zeros((t_dim, b_dim), dtype=np.float64)
    for r in res.results:
        total += r["y"].astype(np.float64)
    disp = total.reshape(-1)[:n] / scale
    peak = np.max(np.abs(disp)) + 1e-8
    return (disp / peak).astype(np.float32)



# revision 2
# speedup vs baseline: 1.0450x; 1.0450x over previous
"""Trainium2 kernel for nn_DifferentiableModalPlate.

displacement[n] = sum_m P_m * exp(-sigma_m*(n-1)*K) * sin(n*omega_m*K) / (sin(omega_m*K)+1e-8)

Each mode is a damped sinusoid Im(A_m * z_m^n) with z_m = r_m*e^{i w_m}.
Splitting n = t*B + j turns the [modes, N] synthesis + mode-reduction into
a single matmul  Y[T, B] = Ut[K, T].T @ W[K, B]  with K = 2*modes rows
(sin/cos pairs):
    Y[t, j] = sum_m  u_m(t)*S_m(j) + v_m(t)*C_m(j)
    u_m(t) = A_m r^(tB) cos(w tB)   S_m(j) = r^j sin(w j)
    v_m(t) = A_m r^(tB) sin(w tB)   C_m(j) = r^j cos(w j)

Only the top-|A| modes are kept (the dropped tail shifts the output well
under the accepted error); the mode axis is sharded across 8 NeuronCores.
Each core streams its K/128 chunks over both HWDGE queues and PSUM-
accumulates one matmul per chunk as soon as that chunk lands; partial
[T, B] outputs are summed on host and normalized by the peak.
"""

import math

import numpy as np

N_CORES = 8
SAMPLE_RATE = 44100
K_DT = 1.0 / SAMPLE_RATE
MAX_OM = 10000.0 * 2.0 * np.pi
MIN_OM = 20.0 * 2.0 * np.pi
LX = 0.5
TAU0, TAU1 = 6.0, 1.0
_OM2 = 2.0 * np.pi * 500.0
_DOMSQ = _OM2 ** 2
ALPHA = float(np.float32(3.0 * np.log(10.0) / _DOMSQ * (_OM2 ** 2 / TAU0)))
BETA = float(np.float32(3.0 * np.log(10.0) / _DOMSQ * (1.0 / TAU1 - 1.0 / TAU0)))
M_MAX = N_MAX = 80
_gm, _gn = np.meshgrid(np.arange(1, M_MAX + 1), np.arange(1, N_MAX + 1), indexing="ij")
M_VEC = _gm.reshape(-1).astype(np.float64)
N_VEC = _gn.reshape(-1).astype(np.float64)

# Cap on synthesized modes: keeping the top-2048 |A| modes moves the output
# by ~3.5e-3 relative (vs the 2e-2 gate), and cuts per-core K from 768 to
# 512 rows (4 fp16 chunks instead of 6).
MAX_MODES = 2048

# Exposed for test harness introspection (exec_time_ns etc.)
LAST_RESULTS = None

_WALRUS_MAX_SEM = 70


def _patch_concourse():
    """Shrink the walrus semaphore space: bass reserves [150,256) assuming
    walrus runs with --max-sem-num=150, but the default compile path never
    passes the flag, so walrus clears/allocates the full space and the NEFF
    epilogue burns ~2us re-zeroing semaphores. Move the boundary to 70."""
    if getattr(_patch_concourse, "_done", False):
        return
    import concourse.bass as _bass
    import concourse.bass_utils as _bu

    _bass.get_kernel_semaphore_range = lambda: range(_WALRUS_MAX_SEM, 256)

    _orig_run_command = _bu.run_command

    def _patched_run_command(argv, **kwargs):
        if argv and "walrus_driver" in str(argv[0]):
            argv = list(argv) + [f"--max-sem-num={_WALRUS_MAX_SEM}"]
        return _orig_run_command(argv, **kwargs)

    _bu.run_command = _patched_run_command

    # If BASS_TRACE is set but the image's antenv lacks axon_hooks,
    # run_bass_kernel_spmd would crash on import; give it a shim that
    # degrades to no trace (or the real ctypes hook when available).
    try:
        import antenv.axon_hooks  # noqa: F401
    except ImportError:
        import sys
        import types

        import antenv

        mod = types.ModuleType("antenv.axon_hooks")
        holder = {"h": None}
        mod.set_axon_ntff_profile_hook = lambda h: holder.__setitem__("h", h)
        mod.get_axon_ntff_profile_hook = lambda: holder["h"]
        sys.modules["antenv.axon_hooks"] = mod
        antenv.axon_hooks = mod
        try:
            sys.path.insert(0, "/root/.axon_site")
            from trn_agent_boot.trn_boot import _ntff_profile_via_ctypes

            hook = _ntff_profile_via_ctypes("/opt/axon/libaxon_pjrt.so")
            if hook is not None:
                mod.set_axon_ntff_profile_hook(hook)
        except Exception:
            pass

    _patch_concourse._done = True


def _softplus(x):
    return np.logaddexp(x, 0.0)


def _mode_params(mu_raw, D_over_mu_raw, T0_over_mu_raw, Ly_raw, xo_raw, yo_raw):
    """Per-mode amplitude A, decay rate r = exp(-sigma*K), phase step w = omega*K (f64)."""
    mu = _softplus(mu_raw) + 1e-4
    D_over_mu = _softplus(D_over_mu_raw) + 1e-4
    T0_over_mu = _softplus(T0_over_mu_raw) + 1e-4
    Ly = 1.1 + (4.0 - 1.1) * ((np.tanh(Ly_raw) + 1.0) / 2.0)
    xo = 0.49 * LX + (1.0 - 0.49) * LX * ((np.tanh(xo_raw) + 1.0) / 2.0)
    yo = 0.51 * Ly + (1.0 - 0.51) * Ly * ((np.tanh(yo_raw) + 1.0) / 2.0)
    xi = 0.1 * LX
    yi = 0.1 * Ly

    pi = np.pi
    g1 = (M_VEC * pi / LX) ** 2 + (N_VEC * pi / Ly) ** 2
    omega = np.sqrt(np.maximum(T0_over_mu * g1 + D_over_mu * g1 * g1, 0.0))
    valid = (omega <= MAX_OM) & (omega >= MIN_OM)

    in_w = np.cos(xi * pi * M_VEC / LX) * np.cos(yi * pi * N_VEC / Ly)
    out_w = np.cos(xo * pi * M_VEC / LX) * np.cos(yo * pi * N_VEC / Ly)
    sigma = ALPHA + BETA * omega ** 2
    ms = 0.25 * mu * LX * Ly
    P = out_w * in_w * (K_DT ** 2) * np.exp(-sigma * K_DT) / ms * valid

    keep = P != 0.0
    P, omega, sigma = P[keep], omega[keep], sigma[keep]
    A = P * np.exp(sigma * K_DT) / (np.sin(omega * K_DT) + 1e-8)
    w = omega * K_DT
    neg_sk = -sigma * K_DT  # log(r)

    # Keep only the strongest MAX_MODES modes by |A|.
    if A.shape[0] > MAX_MODES:
        order = np.argsort(np.abs(A))
        kept = np.sort(order[-MAX_MODES:])
        A, neg_sk, w = A[kept], neg_sk[kept], w[kept]
    return A, neg_sk, w


_PROGRAM_CACHE = {}


def _build_program(n_chunks, t_dim, b_dim):
    """Bass program: Y[t_dim, b_dim] = sum_k UW[:, k, :t].T @ UW[:, k, t:].

    States (first t_dim cols) and tables (last b_dim cols) are packed into
    one fp16 chunk-major tensor [128, n_chunks, t_dim+b_dim]. Each chunk is
    its own SBUF tile fed by its own DMA, alternating between the two HWDGE
    queues (scalar, sync), so matmul k fires as soon as chunk k lands.
    """
    import concourse.tile as tile
    from concourse import bacc, mybir

    nc = bacc.Bacc(
        "TRN2",
        target_bir_lowering=False,
        debug=False,
        enable_asserts=False,
        enable_partition_id=False,
        num_devices=N_CORES,
    )
    f32 = mybir.dt.float32
    f16 = mybir.dt.float16
    uw_d = nc.dram_tensor(
        "uw", [128, n_chunks, t_dim + b_dim], f16, kind="ExternalInput"
    )
    y_d = nc.dram_tensor("y", [t_dim, b_dim], f32, kind="ExternalOutput")

    dma_engines = [nc.scalar, nc.sync]

    with tile.TileContext(nc) as tc:
        with (
            tc.tile_pool(name="pin", bufs=1) as pin,
            tc.tile_pool(name="pps", bufs=1, space="PSUM") as pps,
        ):
            acc = pps.tile([t_dim, b_dim], f32)
            tiles = []
            for k in range(n_chunks):
                gt = pin.tile([128, t_dim + b_dim], f16, tag=f"c{k}")
                dma_engines[k % len(dma_engines)].dma_start(
                    out=gt[:], in_=uw_d[:, k, :]
                )
                tiles.append(gt)
            for k in range(n_chunks):
                nc.tensor.matmul(
                    acc[:],
                    tiles[k][:, 0:t_dim],
                    tiles[k][:, t_dim:t_dim + b_dim],
                    start=(k == 0),
                    stop=(k == n_chunks - 1),
                )
            y_t = pin.tile([t_dim, b_dim], f32, tag="yout")
            nc.vector.tensor_copy(y_t[:], acc[:])
            half = b_dim // 2
            nc.scalar.dma_start(out=y_d[:, 0:half], in_=y_t[:, 0:half])
            nc.sync.dma_start(out=y_d[:, half:b_dim], in_=y_t[:, half:b_dim])
    nc.compile()
    return nc


def kernel(mu_raw, D_over_mu_raw, T0_over_mu_raw, Ly_raw, xo_raw, yo_raw, num_samples):
    global LAST_RESULTS
    _patch_concourse()
    from concourse.bass_utils import run_bass_kernel_spmd

    n = int(num_samples)
    A, neg_sk, w = _mode_params(
        float(mu_raw), float(D_over_mu_raw), float(T0_over_mu_raw),
        float(Ly_raw), float(xo_raw), float(yo_raw),
    )
    nv = A.shape[0]
    if nv == 0 or n == 0:
        return np.zeros(n, dtype=np.float32)

    # Block decomposition: n = t*B + j, T <= 128 (PSUM partitions), B <= 512 (bank).
    b_dim = max(1, math.ceil(n / 128))
    t_dim = math.ceil(n / b_dim)
    assert b_dim <= 512 and t_dim <= 128, (t_dim, b_dim)

    mc = math.ceil(nv / N_CORES)          # modes per core
    kc = ((2 * mc + 127) // 128) * 128    # K rows per core, padded
    n_chunks = kc // 128

    # f64 tables/states for all kept modes at once.
    jj = np.arange(b_dim, dtype=np.float64)
    tt = np.arange(t_dim, dtype=np.float64) * b_dim
    decay_j = np.exp(np.outer(neg_sk, jj))        # [nv, B]
    phase_j = np.outer(w, jj)
    S = (decay_j * np.sin(phase_j)).astype(np.float32)
    C = (decay_j * np.cos(phase_j)).astype(np.float32)
    decay_t = A[:, None] * np.exp(np.outer(neg_sk, tt))  # [nv, T]
    phase_t = np.outer(w, tt)
    U = (decay_t * np.cos(phase_t)).astype(np.float32)
    V = (decay_t * np.sin(phase_t)).astype(np.float32)

    # Global power-of-2 scale so fp16 states stay normal (range ~2e-5 raw).
    # The scale divides out of the partial sums before normalization.
    m_abs = max(np.abs(U).max(), np.abs(V).max(), 1e-300)
    scale = 2.0 ** np.floor(np.log2(16384.0 / m_abs))
    U16 = (U * scale).astype(np.float16)
    V16 = (V * scale).astype(np.float16)
    S16 = S.astype(np.float16)
    C16 = C.astype(np.float16)

    in_maps = []
    for c in range(N_CORES):
        lo, hi = c * mc, min((c + 1) * mc, nv)
        m = hi - lo
        ut = np.zeros((kc, t_dim), dtype=np.float16)
        wt = np.zeros((kc, b_dim), dtype=np.float16)
        if m > 0:
            ut[:m] = U16[lo:hi]
            ut[mc:mc + m] = V16[lo:hi]
            wt[:m] = S16[lo:hi]
            wt[mc:mc + m] = C16[lo:hi]
        # chunk-major pack: [128, n_chunks, t_dim+b_dim], row k=ki*128+p -> [p, ki, :]
        uw = np.concatenate(
            [ut.reshape(n_chunks, 128, t_dim), wt.reshape(n_chunks, 128, b_dim)],
            axis=2,
        ).transpose(1, 0, 2)
        in_maps.append({"uw": np.ascontiguousarray(uw)})

    key = (n_chunks, t_dim, b_dim)
    if key not in _PROGRAM_CACHE:
        _PROGRAM_CACHE[key] = _build_program(*key)
    nc = _PROGRAM_CACHE[key]

    res = run_bass_kernel_spmd(nc, in_maps, core_ids=list(range(N_CORES)))
    LAST_RESULTS = res

    total = np.zeros((t_dim, b_dim), dtype=np.float64)
    for r in res.results:
        total += r["y"].astype(np.float64)
    disp = total.reshape(-1)[:n] / scale
    peak = np.max(np.abs(disp)) + 1e-8
    return (disp / peak).astype(np.float32)


# revision 4
# speedup vs baseline: 1.0647x; 1.0189x over previous
"""Trainium2 kernel for nn_DifferentiableModalPlate.

displacement[n] = sum_m P_m * exp(-sigma_m*(n-1)*K) * sin(n*omega_m*K) / (sin(omega_m*K)+1e-8)

Each mode is a damped sinusoid Im(A_m * z_m^n) with z_m = r_m*e^{i w_m}.
Splitting n = t*B + j turns the [modes, N] synthesis + mode-reduction into
a single matmul  Y[T, B] = Ut[K, T].T @ W[K, B]  with K = 2*modes rows
(sin/cos pairs):
    Y[t, j] = sum_m  u_m(t)*S_m(j) + v_m(t)*C_m(j)
    u_m(t) = A_m r^(tB) cos(w tB)   S_m(j) = r^j sin(w j)
    v_m(t) = A_m r^(tB) sin(w tB)   C_m(j) = r^j cos(w j)

Only the top-|A| modes are kept (the dropped tail shifts the output well
under the accepted error); the mode axis is sharded across 8 NeuronCores.
Each core streams its K/128 chunks over both HWDGE queues and PSUM-
accumulates one matmul per chunk as soon as that chunk lands; partial
[T, B] outputs are summed on host and normalized by the peak.
"""

import math

import numpy as np

N_CORES = 8
SAMPLE_RATE = 44100
K_DT = 1.0 / SAMPLE_RATE
MAX_OM = 10000.0 * 2.0 * np.pi
MIN_OM = 20.0 * 2.0 * np.pi
LX = 0.5
TAU0, TAU1 = 6.0, 1.0
_OM2 = 2.0 * np.pi * 500.0
_DOMSQ = _OM2 ** 2
ALPHA = float(np.float32(3.0 * np.log(10.0) / _DOMSQ * (_OM2 ** 2 / TAU0)))
BETA = float(np.float32(3.0 * np.log(10.0) / _DOMSQ * (1.0 / TAU1 - 1.0 / TAU0)))
M_MAX = N_MAX = 80
_gm, _gn = np.meshgrid(np.arange(1, M_MAX + 1), np.arange(1, N_MAX + 1), indexing="ij")
M_VEC = _gm.reshape(-1).astype(np.float64)
N_VEC = _gn.reshape(-1).astype(np.float64)

# Cap on synthesized modes: keeping the top-2048 |A| modes moves the output
# by ~3.5e-3 relative (vs the 2e-2 gate), and cuts per-core K from 768 to
# 512 rows (4 fp16 chunks instead of 6).
MAX_MODES = 2048

# Exposed for test harness introspection (exec_time_ns etc.)
LAST_RESULTS = None

_WALRUS_MAX_SEM = 70


def _patch_concourse():
    """Shrink the walrus semaphore space: bass reserves [150,256) assuming
    walrus runs with --max-sem-num=150, but the default compile path never
    passes the flag, so walrus clears/allocates the full space and the NEFF
    epilogue burns ~2us re-zeroing semaphores. Move the boundary to 70."""
    if getattr(_patch_concourse, "_done", False):
        return
    import concourse.bass as _bass
    import concourse.bass_utils as _bu

    _bass.get_kernel_semaphore_range = lambda: range(_WALRUS_MAX_SEM, 256)

    _orig_run_command = _bu.run_command

    def _patched_run_command(argv, **kwargs):
        if argv and "walrus_driver" in str(argv[0]):
            argv = list(argv) + [f"--max-sem-num={_WALRUS_MAX_SEM}"]
        return _orig_run_command(argv, **kwargs)

    _bu.run_command = _patched_run_command

    # If BASS_TRACE is set but the image's antenv lacks axon_hooks,
    # run_bass_kernel_spmd would crash on import; give it a shim that
    # degrades to no trace (or the real ctypes hook when available).
    try:
        import antenv.axon_hooks  # noqa: F401
    except ImportError:
        import sys
        import types

        import antenv

        mod = types.ModuleType("antenv.axon_hooks")
        holder = {"h": None}
        mod.set_axon_ntff_profile_hook = lambda h: holder.__setitem__("h", h)
        mod.get_axon_ntff_profile_hook = lambda: holder["h"]
        sys.modules["antenv.axon_hooks"] = mod
        antenv.axon_hooks = mod
        try:
            sys.path.insert(0, "/root/.axon_site")
            from trn_agent_boot.trn_boot import _ntff_profile_via_ctypes

            hook = _ntff_profile_via_ctypes("/opt/axon/libaxon_pjrt.so")
            if hook is not None:
                mod.set_axon_ntff_profile_hook(hook)
        except Exception:
            pass

    _patch_concourse._done = True


def _softplus(x):
    return np.logaddexp(x, 0.0)


def _mode_params(mu_raw, D_over_mu_raw, T0_over_mu_raw, Ly_raw, xo_raw, yo_raw):
    """Per-mode amplitude A, decay rate r = exp(-sigma*K), phase step w = omega*K (f64)."""
    mu = _softplus(mu_raw) + 1e-4
    D_over_mu = _softplus(D_over_mu_raw) + 1e-4
    T0_over_mu = _softplus(T0_over_mu_raw) + 1e-4
    Ly = 1.1 + (4.0 - 1.1) * ((np.tanh(Ly_raw) + 1.0) / 2.0)
    xo = 0.49 * LX + (1.0 - 0.49) * LX * ((np.tanh(xo_raw) + 1.0) / 2.0)
    yo = 0.51 * Ly + (1.0 - 0.51) * Ly * ((np.tanh(yo_raw) + 1.0) / 2.0)
    xi = 0.1 * LX
    yi = 0.1 * Ly

    pi = np.pi
    g1 = (M_VEC * pi / LX) ** 2 + (N_VEC * pi / Ly) ** 2
    omega = np.sqrt(np.maximum(T0_over_mu * g1 + D_over_mu * g1 * g1, 0.0))
    valid = (omega <= MAX_OM) & (omega >= MIN_OM)

    in_w = np.cos(xi * pi * M_VEC / LX) * np.cos(yi * pi * N_VEC / Ly)
    out_w = np.cos(xo * pi * M_VEC / LX) * np.cos(yo * pi * N_VEC / Ly)
    sigma = ALPHA + BETA * omega ** 2
    ms = 0.25 * mu * LX * Ly
    P = out_w * in_w * (K_DT ** 2) * np.exp(-sigma * K_DT) / ms * valid

    keep = P != 0.0
    P, omega, sigma = P[keep], omega[keep], sigma[keep]
    A = P * np.exp(sigma * K_DT) / (np.sin(omega * K_DT) + 1e-8)
    w = omega * K_DT
    neg_sk = -sigma * K_DT  # log(r)

    # Keep only the strongest MAX_MODES modes by |A|.
    if A.shape[0] > MAX_MODES:
        order = np.argsort(np.abs(A))
        kept = np.sort(order[-MAX_MODES:])
        A, neg_sk, w = A[kept], neg_sk[kept], w[kept]
    return A, neg_sk, w


_PROGRAM_CACHE = {}


def _build_program(n_chunks, t_dim, b_dim):
    """Bass program: Y[t_dim, b_dim] = sum_k UW[:, k, :t].T @ UW[:, k, t:].

    States (first t_dim cols) and tables (last b_dim cols) are packed into
    one fp16 chunk-major tensor [128, n_chunks, t_dim+b_dim]. Each chunk is
    its own SBUF tile fed by its own DMA, alternating between the two HWDGE
    queues (scalar, sync), so matmul k fires as soon as chunk k lands.
    """
    import concourse.tile as tile
    from concourse import bacc, mybir

    nc = bacc.Bacc(
        "TRN2",
        target_bir_lowering=False,
        debug=False,
        enable_asserts=False,
        enable_partition_id=False,
        num_devices=N_CORES,
    )
    f32 = mybir.dt.float32
    f16 = mybir.dt.float16
    uw_d = nc.dram_tensor(
        "uw", [128, n_chunks, t_dim + b_dim], f16, kind="ExternalInput"
    )
    y_d = nc.dram_tensor("y", [t_dim, b_dim], f32, kind="ExternalOutput")

    n_warm = 9
    cw = t_dim + b_dim
    assert n_chunks % 2 == 0
    half_c = n_chunks // 2

    with tile.TileContext(nc) as tc:
        with (
            tc.tile_pool(name="pin", bufs=1) as pin,
            tc.tile_pool(name="pps", bufs=1, space="PSUM") as pps,
        ):
            acc = pps.tile([t_dim, b_dim], f32)
            warm_acc = pps.tile([t_dim, b_dim], f32, tag="warm")
            # Pair each HWDGE queue with half the chunks in one DMA: the
            # per-partition run doubles to 2*cw*2 bytes, which doubles the
            # per-queue packet efficiency vs chunk-at-a-time loads.
            ta = pin.tile([128, half_c, cw], f16, tag="ca")
            tb = pin.tile([128, half_c, cw], f16, tag="cb")
            nc.scalar.dma_start(out=ta[:], in_=uw_d[:, 0:half_c, :])
            nc.sync.dma_start(out=tb[:], in_=uw_d[:, half_c:n_chunks, :])
            # Dummy matmuls on an (uninitialized) scratch tile ramp the PE
            # DVFS p-state while the input streams; results land in a dead
            # PSUM bank.
            scratch = pin.tile([128, cw], f16, tag="scratch")
            nc.vector.memset(scratch[:], 0)
            for _ in range(n_warm):
                nc.tensor.matmul(
                    warm_acc[:],
                    scratch[:, 0:t_dim],
                    scratch[:, t_dim:cw],
                    start=True,
                    stop=True,
                )
            for k in range(n_chunks):
                src = ta if k < half_c else tb
                ki = k % half_c
                nc.tensor.matmul(
                    acc[:],
                    src[:, ki, 0:t_dim],
                    src[:, ki, t_dim:cw],
                    start=(k == 0),
                    stop=(k == n_chunks - 1),
                )
            # Split the PSUM drain into halves so the first store overlaps
            # the second copy.
            half_b = (b_dim + 1) // 2
            y_a = pin.tile([t_dim, half_b], f32, tag="ya")
            y_b = pin.tile([t_dim, b_dim - half_b], f32, tag="yb")
            nc.vector.tensor_copy(y_a[:], acc[:, 0:half_b])
            nc.scalar.dma_start(out=y_d[:, 0:half_b], in_=y_a[:])
            nc.vector.tensor_copy(y_b[:], acc[:, half_b:b_dim])
            nc.sync.dma_start(out=y_d[:, half_b:b_dim], in_=y_b[:])
    nc.compile()
    return nc


def kernel(mu_raw, D_over_mu_raw, T0_over_mu_raw, Ly_raw, xo_raw, yo_raw, num_samples):
    global LAST_RESULTS
    _patch_concourse()
    from concourse.bass_utils import run_bass_kernel_spmd

    n = int(num_samples)
    A, neg_sk, w = _mode_params(
        float(mu_raw), float(D_over_mu_raw), float(T0_over_mu_raw),
        float(Ly_raw), float(xo_raw), float(yo_raw),
    )
    nv = A.shape[0]
    if nv == 0 or n == 0:
        return np.zeros(n, dtype=np.float32)

    # Block decomposition: n = t*B + j, T <= 128 (PSUM partitions), B <= 512 (bank).
    b_dim = max(1, math.ceil(n / 128))
    t_dim = math.ceil(n / b_dim)
    assert b_dim <= 512 and t_dim <= 128, (t_dim, b_dim)

    mc = math.ceil(nv / N_CORES)          # modes per core
    kc = ((2 * mc + 127) // 128) * 128    # K rows per core, padded
    n_chunks = kc // 128

    # f64 tables/states for all kept modes at once.
    jj = np.arange(b_dim, dtype=np.float64)
    tt = np.arange(t_dim, dtype=np.float64) * b_dim
    decay_j = np.exp(np.outer(neg_sk, jj))        # [nv, B]
    phase_j = np.outer(w, jj)
    S = (decay_j * np.sin(phase_j)).astype(np.float32)
    C = (decay_j * np.cos(phase_j)).astype(np.float32)
    decay_t = A[:, None] * np.exp(np.outer(neg_sk, tt))  # [nv, T]
    phase_t = np.outer(w, tt)
    U = (decay_t * np.cos(phase_t)).astype(np.float32)
    V = (decay_t * np.sin(phase_t)).astype(np.float32)

    # Global power-of-2 scale so fp16 states stay normal (range ~2e-5 raw).
    # The scale divides out of the partial sums before normalization.
    m_abs = max(np.abs(U).max(), np.abs(V).max(), 1e-300)
    scale = 2.0 ** np.floor(np.log2(16384.0 / m_abs))
    U16 = (U * scale).astype(np.float16)
    V16 = (V * scale).astype(np.float16)
    S16 = S.astype(np.float16)
    C16 = C.astype(np.float16)

    in_maps = []
    for c in range(N_CORES):
        lo, hi = c * mc, min((c + 1) * mc, nv)
        m = hi - lo
        ut = np.zeros((kc, t_dim), dtype=np.float16)
        wt = np.zeros((kc, b_dim), dtype=np.float16)
        if m > 0:
            ut[:m] = U16[lo:hi]
            ut[mc:mc + m] = V16[lo:hi]
            wt[:m] = S16[lo:hi]
            wt[mc:mc + m] = C16[lo:hi]
        # chunk-major pack: [128, n_chunks, t_dim+b_dim], row k=ki*128+p -> [p, ki, :]
        uw = np.concatenate(
            [ut.reshape(n_chunks, 128, t_dim), wt.reshape(n_chunks, 128, b_dim)],
            axis=2,
        ).transpose(1, 0, 2)
        in_maps.append({"uw": np.ascontiguousarray(uw)})

    key = (n_chunks, t_dim, b_dim)
    if key not in _PROGRAM_CACHE:
        _PROGRAM_CACHE[key] = _build_program(*key)
    nc = _PROGRAM_CACHE[key]

    res = run_bass_kernel_spmd(nc, in_maps, core_ids=list(range(N_CORES)))
    LAST_RESULTS = res

    total = np.zeros((t_dim, b_dim), dtype=np.float64)
    for r in res.results:
        total += r["y"].astype(np.float64)
    disp = total.reshape(-1)[:n] / scale
    peak = np.max(np.abs(disp)) + 1e-8
    return (disp / peak).astype(np.float32)
